# revision 8
# baseline (speedup 1.0000x reference)
"""Trainium2 Bass kernel for bidirectional InfoNCE loss + mutual-NN precision/recall.

S = (d0*t) @ (d1*t)^T with t = 1/sqrt(0.1)  (t^2 = 10), N = M = 12288, D = 128.
Outputs: loss_0, loss_1, precision, recall (4 f32 scalars).

Sharding (symmetric, no collectives): core c owns rows [c*1536,(c+1)*1536) of S
(direction A: lse_0/best_0/pos_0) and the same block of S^T (direction B:
lse_1/best_1/pos_1). Each direction needs the full opposite descriptor set,
which is replicated to all cores.

Per [128, 12288] row-tile (12 per block, per direction):
  PE   : 24 f32r matmuls (4x faster than f32, ~tf32 precision)
  ACT  : 6 exp instrs over [128,2048] PSUM (4 banks) -> fp16 E, accum_out
         gives the row-sum per 2048-group (6 partial sums, summed on host)
  DVE  : fold tree (tensor_tensor max, fp16 2x mode): 12x1024 chunks -> fmax[128,1024]
         rm  = tensor_scalar(op1=max accum) over fmax            (4x mode)
         o*  = scalar_tensor_tensor (fmax >= rm) * iota hunt     (offset in 1024-chunk)
         cnt = 12 x tensor_scalar(is_ge rm2, op1=add accum)      (4x mode)
         where rm2 = rm*(1-6e-3) flags near-ties (fp16 quantization + f32r
         matmul error) for exact host-side fixup.
Host decode: rows with total cnt == 1 and valid o* take best = chunk*1024 + o*;
all others (ties/near-ties) are recomputed exactly from the descriptors.
"""

import sys
import numpy as np

for _p in ("/opt/trn_rl_repo",):
    if _p not in sys.path:
        sys.path.insert(0, _p)

N = 12288
D = 128
NCORES = 8
BLK = N // NCORES          # 1536 rows per core
RT = BLK // 128            # 12 row-tiles per block
CH = 512                   # matmul chunk width
HW_ = 1024                 # hunt/count chunk width
NHC = N // HW_             # 12 hunt chunks
GW = 2048                  # exp group width (4 PSUM banks)
NG = N // GW               # 6 exp groups per row-tile
DELTA = 6e-3               # near-tie window (relative, in exp-value space)

_CACHE = {}


def _build():
    import concourse.bacc as bacc
    import concourse.tile as tile
    from concourse import mybir
    from contextlib import ExitStack

    f32 = mybir.dt.float32
    f32r = mybir.dt.float32r
    f16 = mybir.dt.float16
    Exp = mybir.ActivationFunctionType.Exp
    Alu = mybir.AluOpType

    nc = bacc.Bacc(
        "TRN2",
        target_bir_lowering=False,
        debug=False,
        enable_asserts=False,
        num_devices=1,
    )

    def dram_in(name, shape, dt=f32):
        return nc.dram_tensor(name, shape, dt, kind="ExternalInput").ap()

    def dram_out(name, shape, dt=f32):
        return nc.dram_tensor(name, shape, dt, kind="ExternalOutput").ap()

    d0T = dram_in("d0T", [128, N], f32r)          # desc_0^T, replicated (dir1 rhs)
    d1T = dram_in("d1T", [128, N], f32r)          # desc_1^T, replicated (dir0 rhs)
    d0Tblk = dram_in("d0Tblk", [128, BLK], f32r)  # per-core column slice (dir0 lhsT)
    d1Tblk = dram_in("d1Tblk", [128, BLK], f32r)
    iota = dram_in("iota", [128, HW_], f16)       # 1025..2048 replicated per partition

    outs_spec = {}
    for d in (0, 1):
        outs_spec[d] = (
            dram_out(f"rs{d}", [128, RT * NG]),     # row-sum of exp(10*S) per 2048-group
            dram_out(f"cnt{d}", [128, RT * NHC]),   # near-tie counts per 1024-chunk
            dram_out(f"off{d}", [128, RT]),         # in-chunk offset hunt accum
        )

    with tile.TileContext(nc) as tc, ExitStack() as ctx:
        big = ctx.enter_context(tc.tile_pool(name="big", bufs=1))
        psum = ctx.enter_context(tc.tile_pool(name="psum", bufs=2, space="PSUM"))
        epool = ctx.enter_context(tc.tile_pool(name="epool", bufs=3))
        fpool = ctx.enter_context(tc.tile_pool(name="fold", bufs=1))
        spool = ctx.enter_context(tc.tile_pool(name="small", bufs=2))
        stage = ctx.enter_context(tc.tile_pool(name="stage", bufs=1))

        # small inputs first (fast DMAs -> early first matmul); spread across
        # the three HWDGE queues (SP/ACT/DVE) so transfers run in parallel
        d0Tblk_sb = big.tile([128, BLK], f32r, tag="d0Tblk")
        nc.sync.dma_start(d0Tblk_sb[:], d0Tblk[:])
        d1T_sb = big.tile([128, N], f32r, tag="d1T")
        # first piece small so the first matmul group can start ASAP
        nc.scalar.dma_start(d1T_sb[:, :2048], d1T[:, :2048])
        iota_sb = big.tile([128, HW_], f16, tag="iota")
        nc.sync.dma_start(iota_sb[:], iota[:])
        d1Tblk_sb = big.tile([128, BLK], f32r, tag="d1Tblk")
        nc.gpsimd.dma_start(d1Tblk_sb[:], d1Tblk[:])
        qs = [nc.sync, nc.scalar, nc.gpsimd]
        for p in range(5):
            qs[p % 3].dma_start(d1T_sb[:, 2048 + p * 2048:2048 + (p + 1) * 2048],
                                d1T[:, 2048 + p * 2048:2048 + (p + 1) * 2048])
        d0T_sb = big.tile([128, N], f32r, tag="d0T")
        for p in range(4):
            qs[p % 3].dma_start(d0T_sb[:, p * 3072:(p + 1) * 3072],
                                d0T[:, p * 3072:(p + 1) * 3072])
        # preload the Exp activation table while input DMAs stream
        warm = spool.tile([128, 1], f32, tag="warm")
        nc.vector.memset(warm[:], 0.0)
        warm2 = spool.tile([128, 1], f32, tag="warm2")
        nc.scalar.activation(warm2[:], warm[:], Exp)

        # fold scratch: A [128, 6144] (levels 1,3,4 + dummy zones), B [128, 3072]
        A = fpool.tile([128, 6144], f16, tag="foldA")
        B = fpool.tile([128, 3072], f16, tag="foldB")

        for d in (0, 1):
            lhsT_all = d0Tblk_sb if d == 0 else d1Tblk_sb
            rhs_all = d1T_sb if d == 0 else d0T_sb
            rs_dram, cnt_dram, off_dram = outs_spec[d]

            rs_st = stage.tile([128, RT * NG], f32, tag=f"rs_st{d}")
            cnt_st = stage.tile([128, RT * NHC], f32, tag=f"cnt_st{d}")
            off_st = stage.tile([128, RT], f32, tag=f"off_st{d}")

            for m in range(RT):
                lhsT = lhsT_all[:, m * 128:(m + 1) * 128]
                E = epool.tile([128, N], f16, tag="E")
                for g in range(NG):
                    ps = psum.tile([128, GW], f32, tag="ps")
                    for k in range(4):
                        f = g * 4 + k
                        nc.tensor.matmul(
                            ps[:, k * CH:(k + 1) * CH],
                            lhsT,
                            rhs_all[:, f * CH:(f + 1) * CH],
                            start=True,
                            stop=True,
                        )
                    nc.scalar.activation(
                        E[:, g * GW:(g + 1) * GW],
                        ps[:],
                        Exp,
                        scale=10.0,
                        accum_out=rs_st[:, m * NG + g : m * NG + g + 1],
                    )
                # fold tree: 12 x 1024 chunks -> fmax [128, 1024] in A[:, 1024:2048]
                nc.vector.tensor_tensor(A[:, :6144], E[:, :6144], E[:, 6144:], Alu.max)
                nc.vector.tensor_tensor(B[:, :3072], A[:, :3072], A[:, 3072:6144], Alu.max)
                nc.vector.tensor_tensor(A[:, :1024], B[:, :1024], B[:, 2048:3072], Alu.max)
                nc.vector.tensor_tensor(A[:, 1024:2048], A[:, :1024], B[:, 1024:2048], Alu.max)
                fmax = A[:, 1024:2048]
                # rm = row max (exact fp16 value, f32 accum)
                rm = spool.tile([128, 1], f32, tag="rm")
                nc.vector.tensor_scalar(
                    A[:, 2048:3072], fmax, 1.0, None, Alu.mult, Alu.max,
                    accum_out=rm[:],
                )
                # rm2 = rm*(1-delta): near-tie count threshold
                rm2 = spool.tile([128, 1], f32, tag="rm2")
                nc.vector.tensor_scalar_mul(rm2[:], rm[:], 1.0 - DELTA)
                # offset hunt on folded fmax
                nc.vector.scalar_tensor_tensor(
                    out=A[:, 3072:4096],
                    in0=fmax,
                    scalar=rm[:],
                    in1=iota_sb[:],
                    op0=Alu.is_ge,
                    op1=Alu.mult,
                    accum_out=off_st[:, m:m + 1],
                )
                # per-1024-chunk near-tie counts
                for f in range(NHC):
                    nc.vector.tensor_scalar(
                        A[:, 4096:5120],
                        E[:, f * HW_:(f + 1) * HW_],
                        rm2[:],
                        None,
                        Alu.is_ge,
                        Alu.add,
                        accum_out=cnt_st[:, m * NHC + f : m * NHC + f + 1],
                    )

            nc.sync.dma_start(rs_dram[:], rs_st[:])
            nc.sync.dma_start(cnt_dram[:], cnt_st[:])
            nc.sync.dma_start(off_dram[:], off_st[:])

    nc.compile()
    return nc


def _get_nc():
    if "nc" not in _CACHE:
        _CACHE["nc"] = _build()
    return _CACHE["nc"]


def _tiles(x_blk):
    """[1536, 128] rows -> [128, 1536] partition-major tile layout."""
    return np.ascontiguousarray(
        x_blk.reshape(RT, 128, D).transpose(1, 0, 2).reshape(128, RT * D)
    )


def _unstage(a):
    """[128, RT] staged column-per-row-tile -> [1536] block vector."""
    return np.ascontiguousarray(a.T).reshape(BLK)


def kernel(desc_0, desc_1, corr_0, corr_1, logits_0, logits_1):
    from concourse import bass_utils

    nc = _get_nc()

    d0 = np.asarray(desc_0, dtype=np.float32)
    d1 = np.asarray(desc_1, dtype=np.float32)
    c0 = np.asarray(corr_0)
    c1 = np.asarray(corr_1)
    l0g = np.asarray(logits_0, dtype=np.float32)
    l1g = np.asarray(logits_1, dtype=np.float32)

    d0T = np.ascontiguousarray(d0.T)
    d1T = np.ascontiguousarray(d1.T)
    i0 = np.clip(c0, 0, None).astype(np.int64)
    i1 = np.clip(c1, 0, None).astype(np.int64)
    G0 = d1[i0]   # [N, D]
    G1 = d0[i1]
    # Offset ramp: single match -> accum in [1025, 2048]; k>=2 matches sum to
    # >= 2051, disjoint, so multi-match ambiguity is detectable on the host.
    iota = np.broadcast_to(
        (np.arange(1, HW_ + 1, dtype=np.float16) + np.float16(1024.0))[None, :],
        (128, HW_),
    ).copy()

    in_maps = []
    for c in range(NCORES):
        sl = slice(c * BLK, (c + 1) * BLK)
        in_maps.append({
            "d0T": d0T,
            "d1T": d1T,
            "d0Tblk": np.ascontiguousarray(d0T[:, sl]),
            "d1Tblk": np.ascontiguousarray(d1T[:, sl]),
            "iota": iota,
        })

    import os
    res = bass_utils.run_bass_kernel_spmd(
        nc, in_maps, core_ids=list(range(NCORES)),
        trace=bool(os.environ.get("KERNEL_TRACE")),
    )
    _CACHE["last_res"] = res
    outs = res.results

    rs = {0: [], 1: []}
    best = {0: [], 1: []}
    fixup = {0: [], 1: []}   # global rows needing exact recompute
    for c in range(NCORES):
        o = outs[c]
        for d in (0, 1):
            r6 = o[f"rs{d}"].reshape(128, RT, NG).sum(axis=2, dtype=np.float64)
            rs[d].append(np.ascontiguousarray(r6.T).reshape(BLK))
            cnt = o[f"cnt{d}"].reshape(128, RT, NHC)
            off = o[f"off{d}"]                      # [128, RT]
            total = cnt.sum(axis=2)
            wc = np.argmax(cnt, axis=2)             # winning chunk (count==1 rows)
            oin = off - 1025.0                      # in-chunk offset
            b = wc.astype(np.int64) * HW_ + np.clip(oin, 0, HW_ - 1).astype(np.int64)
            best[d].append(_unstage(b))
            bad = (total != 1.0) | (off < 1024.5) | (off > 2048.5)
            for r in np.nonzero(_unstage(bad))[0]:
                fixup[d].append(c * BLK + int(r))

    rs0 = np.concatenate(rs[0]); rs1 = np.concatenate(rs[1])
    pos_0 = (np.float32(10.0) * np.einsum('ij,ij->i', d0, G0)).astype(np.float32)
    pos_1 = (np.float32(10.0) * np.einsum('ij,ij->i', d1, G1)).astype(np.float32)
    best_0 = np.concatenate(best[0]); best_1 = np.concatenate(best[1])

    # Exact fixup: rows where the max was ambiguous at fp16/f32r precision.
    # Recompute those rows in f64 and take the first argmax (jnp semantics).
    if fixup[0]:
        rows = np.asarray(fixup[0], dtype=np.int64)
        sl = d1.astype(np.float64) @ d0[rows].astype(np.float64).T   # [N, R]
        best_0[rows] = np.argmax(sl, axis=0)
    if fixup[1]:
        rows = np.asarray(fixup[1], dtype=np.int64)
        sl = d0.astype(np.float64) @ d1[rows].astype(np.float64).T
        best_1[rows] = np.argmax(sl, axis=0)

    lse_0 = np.log(rs0).astype(np.float32)
    lse_1 = np.log(rs1).astype(np.float32)

    m0 = c0 >= 0
    m1 = c1 >= 0
    l0 = np.where(m0, lse_0 - pos_0, np.float32(0.0)).astype(np.float32)
    l1 = np.where(m1, lse_1 - pos_1, np.float32(0.0)).astype(np.float32)
    n0 = max(int(m0.sum()), 1)
    n1 = max(int(m1.sum()), 1)
    loss_0 = np.float32(l0.sum(dtype=np.float32) / np.float32(n0))
    loss_1 = np.float32(l1.sum(dtype=np.float32) / np.float32(n1))

    best_0 = np.clip(best_0, 0, N - 1)
    best_1 = np.clip(best_1, 0, N - 1)
    _CACHE["dbg"] = dict(best_0=best_0, best_1=best_1, lse_0=lse_0, lse_1=lse_1,
                         n_fixup=(len(fixup[0]), len(fixup[1])))
    mutual = best_1[best_0] == np.arange(N)
    kp0 = l0g >= 0.0
    kp1 = l1g >= 0.0
    predicted = mutual & kp0 & kp1[best_0]
    correct = (best_0 == c0) & m0
    tp = int((correct & predicted).sum())
    precision = np.float32(np.float32(tp) / np.float32(max(int(predicted.sum()), 1)))
    recall = np.float32(np.float32(tp) / np.float32(n0))

    return loss_0, loss_1, precision, recall


# revision 9
# speedup vs baseline: 1.0037x; 1.0037x over previous
"""Trainium2 Bass kernel for bidirectional InfoNCE loss + mutual-NN precision/recall.

S = (d0*t) @ (d1*t)^T with t = 1/sqrt(0.1)  (t^2 = 10), N = M = 12288, D = 128.
Outputs: loss_0, loss_1, precision, recall (4 f32 scalars).

Sharding (symmetric, no collectives): core c owns rows [c*1536,(c+1)*1536) of S
(direction A: lse_0/best_0/pos_0) and the same block of S^T (direction B:
lse_1/best_1/pos_1). Each direction needs the full opposite descriptor set,
which is replicated to all cores.

Per [128, 12288] row-tile (12 per block, per direction):
  PE   : 24 f32r matmuls (4x faster than f32, ~tf32 precision)
  ACT  : 6 exp instrs over [128,2048] PSUM (4 banks) -> fp16 E, accum_out
         gives the row-sum per 2048-group (6 partial sums, summed on host)
  DVE  : fold tree (tensor_tensor max, fp16 2x mode): 12x1024 chunks -> fmax[128,1024]
         rm  = tensor_scalar(op1=max accum) over fmax            (4x mode)
         o*  = scalar_tensor_tensor (fmax >= rm) * iota hunt     (offset in 1024-chunk)
         cnt = 12 x tensor_scalar(is_ge rm2, op1=add accum)      (4x mode)
         where rm2 = rm*(1-6e-3) flags near-ties (fp16 quantization + f32r
         matmul error) for exact host-side fixup.
Host decode: rows with total cnt == 1 and valid o* take best = chunk*1024 + o*;
all others (ties/near-ties) are recomputed exactly from the descriptors.
"""

import sys
import numpy as np

for _p in ("/opt/trn_rl_repo",):
    if _p not in sys.path:
        sys.path.insert(0, _p)

N = 12288
D = 128
NCORES = 8
BLK = N // NCORES          # 1536 rows per core
RT = BLK // 128            # 12 row-tiles per block
CH = 512                   # matmul chunk width
HW_ = 1024                 # hunt/count chunk width
NHC = N // HW_             # 12 hunt chunks
GW = 2048                  # exp group width (4 PSUM banks)
NG = N // GW               # 6 exp groups per row-tile
DELTA = 6e-3               # near-tie window (relative, in exp-value space)

_CACHE = {}


def _build():
    import concourse.bacc as bacc
    import concourse.tile as tile
    from concourse import mybir
    from contextlib import ExitStack

    f32 = mybir.dt.float32
    f32r = mybir.dt.float32r
    f16 = mybir.dt.float16
    Exp = mybir.ActivationFunctionType.Exp
    Alu = mybir.AluOpType

    nc = bacc.Bacc(
        "TRN2",
        target_bir_lowering=False,
        debug=False,
        enable_asserts=False,
        num_devices=1,
    )

    def dram_in(name, shape, dt=f32):
        return nc.dram_tensor(name, shape, dt, kind="ExternalInput").ap()

    def dram_out(name, shape, dt=f32):
        return nc.dram_tensor(name, shape, dt, kind="ExternalOutput").ap()

    d0T = dram_in("d0T", [128, N], f32r)          # desc_0^T, replicated (dir1 rhs)
    d1T = dram_in("d1T", [128, N], f32r)          # desc_1^T, replicated (dir0 rhs)
    d0Tblk = dram_in("d0Tblk", [128, BLK], f32r)  # per-core column slice (dir0 lhsT)
    d1Tblk = dram_in("d1Tblk", [128, BLK], f32r)
    iota = dram_in("iota", [128, HW_], f16)       # 1025..2048 replicated per partition

    outs_spec = {}
    for d in (0, 1):
        outs_spec[d] = (
            dram_out(f"rs{d}", [128, RT * NG]),     # row-sum of exp(10*S) per 2048-group
            dram_out(f"cnt{d}", [128, RT * NHC]),   # near-tie counts per 1024-chunk
            dram_out(f"off{d}", [128, RT]),         # in-chunk offset hunt accum
        )

    with tile.TileContext(nc) as tc, ExitStack() as ctx:
        big = ctx.enter_context(tc.tile_pool(name="big", bufs=1))
        psum = ctx.enter_context(tc.tile_pool(name="psum", bufs=2, space="PSUM"))
        epool = ctx.enter_context(tc.tile_pool(name="epool", bufs=3))
        fpool = ctx.enter_context(tc.tile_pool(name="fold", bufs=1))
        spool = ctx.enter_context(tc.tile_pool(name="small", bufs=2))
        stage = ctx.enter_context(tc.tile_pool(name="stage", bufs=1))

        # small inputs first (fast DMAs -> early first matmul); spread across
        # the three HWDGE queues (SP/ACT/DVE) so transfers run in parallel
        d0Tblk_sb = big.tile([128, BLK], f32r, tag="d0Tblk")
        nc.sync.dma_start(d0Tblk_sb[:], d0Tblk[:])
        d1T_sb = big.tile([128, N], f32r, tag="d1T")
        # first piece small so the first matmul group can start ASAP
        nc.gpsimd.dma_start(d1T_sb[:, :2048], d1T[:, :2048])
        iota_sb = big.tile([128, HW_], f16, tag="iota")
        nc.sync.dma_start(iota_sb[:], iota[:])
        d1Tblk_sb = big.tile([128, BLK], f32r, tag="d1Tblk")
        nc.gpsimd.dma_start(d1Tblk_sb[:], d1Tblk[:])
        qs = [nc.sync, nc.gpsimd]
        for p in range(5):
            qs[p % 2].dma_start(d1T_sb[:, 2048 + p * 2048:2048 + (p + 1) * 2048],
                                d1T[:, 2048 + p * 2048:2048 + (p + 1) * 2048])
        d0T_sb = big.tile([128, N], f32r, tag="d0T")
        for p in range(4):
            qs[p % 2].dma_start(d0T_sb[:, p * 3072:(p + 1) * 3072],
                                d0T[:, p * 3072:(p + 1) * 3072])
        # preload the Exp activation table while input DMAs stream
        warm = spool.tile([128, 1], f32, tag="warm")
        nc.vector.memset(warm[:], 0.0)
        warm2 = spool.tile([128, 1], f32, tag="warm2")
        nc.scalar.activation(warm2[:], warm[:], Exp)

        # fold scratch: A [128, 6144] (levels 1,3,4 + dummy zones), B [128, 3072]
        A = fpool.tile([128, 6144], f16, tag="foldA")
        B = fpool.tile([128, 3072], f16, tag="foldB")

        for d in (0, 1):
            lhsT_all = d0Tblk_sb if d == 0 else d1Tblk_sb
            rhs_all = d1T_sb if d == 0 else d0T_sb
            rs_dram, cnt_dram, off_dram = outs_spec[d]

            rs_st = stage.tile([128, RT * NG], f32, tag=f"rs_st{d}")
            cnt_st = stage.tile([128, RT * NHC], f32, tag=f"cnt_st{d}")
            off_st = stage.tile([128, RT], f32, tag=f"off_st{d}")

            for m in range(RT):
                lhsT = lhsT_all[:, m * 128:(m + 1) * 128]
                E = epool.tile([128, N], f16, tag="E")
                for g in range(NG):
                    ps = psum.tile([128, GW], f32, tag="ps")
                    for k in range(4):
                        f = g * 4 + k
                        nc.tensor.matmul(
                            ps[:, k * CH:(k + 1) * CH],
                            lhsT,
                            rhs_all[:, f * CH:(f + 1) * CH],
                            start=True,
                            stop=True,
                        )
                    nc.scalar.activation(
                        E[:, g * GW:(g + 1) * GW],
                        ps[:],
                        Exp,
                        scale=10.0,
                        accum_out=rs_st[:, m * NG + g : m * NG + g + 1],
                    )
                # fold tree: 12 x 1024 chunks -> fmax [128, 1024] in A[:, 1024:2048]
                nc.vector.tensor_tensor(A[:, :6144], E[:, :6144], E[:, 6144:], Alu.max)
                nc.vector.tensor_tensor(B[:, :3072], A[:, :3072], A[:, 3072:6144], Alu.max)
                nc.vector.tensor_tensor(A[:, :1024], B[:, :1024], B[:, 2048:3072], Alu.max)
                nc.vector.tensor_tensor(A[:, 1024:2048], A[:, :1024], B[:, 1024:2048], Alu.max)
                fmax = A[:, 1024:2048]
                # rm = row max (exact fp16 value, f32 accum)
                rm = spool.tile([128, 1], f32, tag="rm")
                nc.vector.tensor_scalar(
                    A[:, 2048:3072], fmax, 1.0, None, Alu.mult, Alu.max,
                    accum_out=rm[:],
                )
                # rm2 = rm*(1-delta): near-tie count threshold
                rm2 = spool.tile([128, 1], f32, tag="rm2")
                nc.vector.tensor_scalar_mul(rm2[:], rm[:], 1.0 - DELTA)
                # offset hunt on folded fmax
                nc.vector.scalar_tensor_tensor(
                    out=A[:, 3072:4096],
                    in0=fmax,
                    scalar=rm[:],
                    in1=iota_sb[:],
                    op0=Alu.is_ge,
                    op1=Alu.mult,
                    accum_out=off_st[:, m:m + 1],
                )
                # per-1024-chunk near-tie counts
                for f in range(NHC):
                    nc.vector.tensor_scalar(
                        A[:, 4096:5120],
                        E[:, f * HW_:(f + 1) * HW_],
                        rm2[:],
                        None,
                        Alu.is_ge,
                        Alu.add,
                        accum_out=cnt_st[:, m * NHC + f : m * NHC + f + 1],
                    )

            nc.sync.dma_start(rs_dram[:], rs_st[:])
            nc.sync.dma_start(cnt_dram[:], cnt_st[:])
            nc.sync.dma_start(off_dram[:], off_st[:])

    nc.compile()
    return nc


def _get_nc():
    if "nc" not in _CACHE:
        _CACHE["nc"] = _build()
    return _CACHE["nc"]


def _tiles(x_blk):
    """[1536, 128] rows -> [128, 1536] partition-major tile layout."""
    return np.ascontiguousarray(
        x_blk.reshape(RT, 128, D).transpose(1, 0, 2).reshape(128, RT * D)
    )


def _unstage(a):
    """[128, RT] staged column-per-row-tile -> [1536] block vector."""
    return np.ascontiguousarray(a.T).reshape(BLK)


def kernel(desc_0, desc_1, corr_0, corr_1, logits_0, logits_1):
    from concourse import bass_utils

    nc = _get_nc()

    d0 = np.asarray(desc_0, dtype=np.float32)
    d1 = np.asarray(desc_1, dtype=np.float32)
    c0 = np.asarray(corr_0)
    c1 = np.asarray(corr_1)
    l0g = np.asarray(logits_0, dtype=np.float32)
    l1g = np.asarray(logits_1, dtype=np.float32)

    d0T = np.ascontiguousarray(d0.T)
    d1T = np.ascontiguousarray(d1.T)
    i0 = np.clip(c0, 0, None).astype(np.int64)
    i1 = np.clip(c1, 0, None).astype(np.int64)
    G0 = d1[i0]   # [N, D]
    G1 = d0[i1]
    # Offset ramp: single match -> accum in [1025, 2048]; k>=2 matches sum to
    # >= 2051, disjoint, so multi-match ambiguity is detectable on the host.
    iota = np.broadcast_to(
        (np.arange(1, HW_ + 1, dtype=np.float16) + np.float16(1024.0))[None, :],
        (128, HW_),
    ).copy()

    in_maps = []
    for c in range(NCORES):
        sl = slice(c * BLK, (c + 1) * BLK)
        in_maps.append({
            "d0T": d0T,
            "d1T": d1T,
            "d0Tblk": np.ascontiguousarray(d0T[:, sl]),
            "d1Tblk": np.ascontiguousarray(d1T[:, sl]),
            "iota": iota,
        })

    import os
    res = bass_utils.run_bass_kernel_spmd(
        nc, in_maps, core_ids=list(range(NCORES)),
        trace=bool(os.environ.get("KERNEL_TRACE")),
    )
    _CACHE["last_res"] = res
    outs = res.results

    rs = {0: [], 1: []}
    best = {0: [], 1: []}
    fixup = {0: [], 1: []}   # global rows needing exact recompute
    for c in range(NCORES):
        o = outs[c]
        for d in (0, 1):
            r6 = o[f"rs{d}"].reshape(128, RT, NG).sum(axis=2, dtype=np.float64)
            rs[d].append(np.ascontiguousarray(r6.T).reshape(BLK))
            cnt = o[f"cnt{d}"].reshape(128, RT, NHC)
            off = o[f"off{d}"]                      # [128, RT]
            total = cnt.sum(axis=2)
            wc = np.argmax(cnt, axis=2)             # winning chunk (count==1 rows)
            oin = off - 1025.0                      # in-chunk offset
            b = wc.astype(np.int64) * HW_ + np.clip(oin, 0, HW_ - 1).astype(np.int64)
            best[d].append(_unstage(b))
            bad = (total != 1.0) | (off < 1024.5) | (off > 2048.5)
            for r in np.nonzero(_unstage(bad))[0]:
                fixup[d].append(c * BLK + int(r))

    rs0 = np.concatenate(rs[0]); rs1 = np.concatenate(rs[1])
    pos_0 = (np.float32(10.0) * np.einsum('ij,ij->i', d0, G0)).astype(np.float32)
    pos_1 = (np.float32(10.0) * np.einsum('ij,ij->i', d1, G1)).astype(np.float32)
    best_0 = np.concatenate(best[0]); best_1 = np.concatenate(best[1])

    # Exact fixup: rows where the max was ambiguous at fp16/f32r precision.
    # Recompute those rows in f64 and take the first argmax (jnp semantics).
    if fixup[0]:
        rows = np.asarray(fixup[0], dtype=np.int64)
        sl = d1.astype(np.float64) @ d0[rows].astype(np.float64).T   # [N, R]
        best_0[rows] = np.argmax(sl, axis=0)
    if fixup[1]:
        rows = np.asarray(fixup[1], dtype=np.int64)
        sl = d0.astype(np.float64) @ d1[rows].astype(np.float64).T
        best_1[rows] = np.argmax(sl, axis=0)

    lse_0 = np.log(rs0).astype(np.float32)
    lse_1 = np.log(rs1).astype(np.float32)

    m0 = c0 >= 0
    m1 = c1 >= 0
    l0 = np.where(m0, lse_0 - pos_0, np.float32(0.0)).astype(np.float32)
    l1 = np.where(m1, lse_1 - pos_1, np.float32(0.0)).astype(np.float32)
    n0 = max(int(m0.sum()), 1)
    n1 = max(int(m1.sum()), 1)
    loss_0 = np.float32(l0.sum(dtype=np.float32) / np.float32(n0))
    loss_1 = np.float32(l1.sum(dtype=np.float32) / np.float32(n1))

    best_0 = np.clip(best_0, 0, N - 1)
    best_1 = np.clip(best_1, 0, N - 1)
    _CACHE["dbg"] = dict(best_0=best_0, best_1=best_1, lse_0=lse_0, lse_1=lse_1,
                         n_fixup=(len(fixup[0]), len(fixup[1])))
    mutual = best_1[best_0] == np.arange(N)
    kp0 = l0g >= 0.0
    kp1 = l1g >= 0.0
    predicted = mutual & kp0 & kp1[best_0]
    correct = (best_0 == c0) & m0
    tp = int((correct & predicted).sum())
    precision = np.float32(np.float32(tp) / np.float32(max(int(predicted.sum()), 1)))
    recall = np.float32(np.float32(tp) / np.float32(n0))

    return loss_0, loss_1, precision, recall


# revision 11
# speedup vs baseline: 1.0163x; 1.0125x over previous
"""Trainium2 Bass kernel for bidirectional InfoNCE loss + mutual-NN precision/recall.

S = (d0*t) @ (d1*t)^T with t = 1/sqrt(0.1)  (t^2 = 10), N = M = 12288, D = 128.
Outputs: loss_0, loss_1, precision, recall (4 f32 scalars).

Sharding (symmetric, no collectives): core c owns rows [c*1536,(c+1)*1536) of S
(direction A: lse_0/best_0/pos_0) and the same block of S^T (direction B:
lse_1/best_1/pos_1). Each direction needs the full opposite descriptor set,
which is replicated to all cores.

Per [128, 12288] row-tile (12 per block, per direction):
  PE   : 24 f32r matmuls (4x faster than f32, ~tf32 precision)
  ACT  : 6 exp instrs over [128,2048] PSUM (4 banks) -> fp16 E, accum_out
         gives the row-sum per 2048-group (6 partial sums, summed on host)
  DVE  : fold tree (tensor_tensor max, fp16 2x mode): 12x1024 chunks -> fmax[128,1024]
         rm  = tensor_scalar(op1=max accum) over fmax            (4x mode)
         o*  = scalar_tensor_tensor (fmax >= rm) * iota hunt     (offset in 1024-chunk)
         cnt = 12 x tensor_scalar(is_ge rm2, op1=add accum)      (4x mode)
         where rm2 = rm*(1-6e-3) flags near-ties (fp16 quantization + f32r
         matmul error) for exact host-side fixup.
Host decode: rows with total cnt == 1 and valid o* take best = chunk*1024 + o*;
all others (ties/near-ties) are recomputed exactly from the descriptors.
"""

import sys
import numpy as np

for _p in ("/opt/trn_rl_repo",):
    if _p not in sys.path:
        sys.path.insert(0, _p)

N = 12288
D = 128
NCORES = 8
BLK = N // NCORES          # 1536 rows per core
RT = BLK // 128            # 12 row-tiles per block
CH = 512                   # matmul chunk width
HW_ = 1024                 # hunt/count chunk width
NHC = N // HW_             # 12 hunt chunks
GW = 2048                  # exp group width (4 PSUM banks)
NG = N // GW               # 6 exp groups per row-tile
DELTA = 6e-3               # near-tie window (relative, in exp-value space)

_CACHE = {}


def _build():
    import concourse.bacc as bacc
    import concourse.tile as tile
    from concourse import mybir
    from contextlib import ExitStack

    f32 = mybir.dt.float32
    f32r = mybir.dt.float32r
    f16 = mybir.dt.float16
    Exp = mybir.ActivationFunctionType.Exp
    Alu = mybir.AluOpType

    nc = bacc.Bacc(
        "TRN2",
        target_bir_lowering=False,
        debug=False,
        enable_asserts=False,
        num_devices=1,
    )

    def dram_in(name, shape, dt=f32):
        return nc.dram_tensor(name, shape, dt, kind="ExternalInput").ap()

    def dram_out(name, shape, dt=f32):
        return nc.dram_tensor(name, shape, dt, kind="ExternalOutput").ap()

    d0T = dram_in("d0T", [128, N], f32r)          # desc_0^T, replicated (dir1 rhs)
    d1T = dram_in("d1T", [128, N], f32r)          # desc_1^T, replicated (dir0 rhs)
    d0Tblk = dram_in("d0Tblk", [128, BLK], f32r)  # per-core column slice (dir0 lhsT)
    d1Tblk = dram_in("d1Tblk", [128, BLK], f32r)
    iota = dram_in("iota", [128, HW_], f16)       # 1025..2048 replicated per partition

    outs_spec = {}
    for d in (0, 1):
        outs_spec[d] = (
            dram_out(f"rs{d}", [128, RT * NG]),     # row-sum of exp(10*S) per 2048-group
            dram_out(f"cnt{d}", [128, RT * NHC]),   # near-tie counts per 1024-chunk
            dram_out(f"off{d}", [128, RT]),         # in-chunk offset hunt accum
        )

    with tile.TileContext(nc) as tc, ExitStack() as ctx:
        big = ctx.enter_context(tc.tile_pool(name="big", bufs=1))
        psum = ctx.enter_context(tc.tile_pool(name="psum", bufs=2, space="PSUM"))
        epool = ctx.enter_context(tc.tile_pool(name="epool", bufs=3))
        fpool = ctx.enter_context(tc.tile_pool(name="fold", bufs=1))
        spool = ctx.enter_context(tc.tile_pool(name="small", bufs=2))
        stage = ctx.enter_context(tc.tile_pool(name="stage", bufs=1))

        # small inputs first (fast DMAs -> early first matmul); spread across
        # the three HWDGE queues (SP/ACT/DVE) so transfers run in parallel
        d0Tblk_sb = big.tile([128, BLK], f32r, tag="d0Tblk")
        nc.sync.dma_start(d0Tblk_sb[:], d0Tblk[:])
        d1T_sb = big.tile([128, N], f32r, tag="d1T")
        # first pieces small so the first matmul groups can start ASAP
        nc.gpsimd.dma_start(d1T_sb[:, :1024], d1T[:, :1024])
        nc.sync.dma_start(d1T_sb[:, 1024:2048], d1T[:, 1024:2048])
        iota_sb = big.tile([128, HW_], f16, tag="iota")
        nc.sync.dma_start(iota_sb[:], iota[:])
        d1Tblk_sb = big.tile([128, BLK], f32r, tag="d1Tblk")
        nc.gpsimd.dma_start(d1Tblk_sb[:], d1Tblk[:])
        qs = [nc.sync, nc.gpsimd]
        for p in range(5):
            qs[p % 2].dma_start(d1T_sb[:, 2048 + p * 2048:2048 + (p + 1) * 2048],
                                d1T[:, 2048 + p * 2048:2048 + (p + 1) * 2048])
        d0T_sb = big.tile([128, N], f32r, tag="d0T")
        for p in range(4):
            qs[p % 2].dma_start(d0T_sb[:, p * 3072:(p + 1) * 3072],
                                d0T[:, p * 3072:(p + 1) * 3072])
        # preload the Exp activation table while input DMAs stream
        warm = spool.tile([128, 1], f32, tag="warm")
        nc.vector.memset(warm[:], 0.0)
        warm2 = spool.tile([128, 1], f32, tag="warm2")
        nc.scalar.activation(warm2[:], warm[:], Exp)

        # fold scratch: A [128, 6144] (levels 1,3,4 + dummy zones), B [128, 3072]
        A = fpool.tile([128, 6144], f16, tag="foldA")
        B = fpool.tile([128, 3072], f16, tag="foldB")

        for d in (0, 1):
            lhsT_all = d0Tblk_sb if d == 0 else d1Tblk_sb
            rhs_all = d1T_sb if d == 0 else d0T_sb
            rs_dram, cnt_dram, off_dram = outs_spec[d]

            rs_st = stage.tile([128, RT * NG], f32, tag=f"rs_st{d}")
            cnt_st = stage.tile([128, RT * NHC], f32, tag=f"cnt_st{d}")
            off_st = stage.tile([128, RT], f32, tag=f"off_st{d}")

            for m in range(RT):
                lhsT = lhsT_all[:, m * 128:(m + 1) * 128]
                E = epool.tile([128, N], f16, tag="E")
                for g in range(NG):
                    ps = psum.tile([128, GW], f32, tag="ps")
                    for k in range(4):
                        f = g * 4 + k
                        nc.tensor.matmul(
                            ps[:, k * CH:(k + 1) * CH],
                            lhsT,
                            rhs_all[:, f * CH:(f + 1) * CH],
                            start=True,
                            stop=True,
                        )
                    nc.scalar.activation(
                        E[:, g * GW:(g + 1) * GW],
                        ps[:],
                        Exp,
                        scale=10.0,
                        accum_out=rs_st[:, m * NG + g : m * NG + g + 1],
                    )
                # fold tree: 12 x 1024 chunks -> fmax [128, 1024] in A[:, 1024:2048]
                nc.vector.tensor_tensor(A[:, :6144], E[:, :6144], E[:, 6144:], Alu.max)
                nc.vector.tensor_tensor(B[:, :3072], A[:, :3072], A[:, 3072:6144], Alu.max)
                nc.vector.tensor_tensor(A[:, :1024], B[:, :1024], B[:, 2048:3072], Alu.max)
                nc.vector.tensor_tensor(A[:, 1024:2048], A[:, :1024], B[:, 1024:2048], Alu.max)
                fmax = A[:, 1024:2048]
                # rm2 = rowmax*(1-delta) in one op: accum = max(fmax*(1-delta)).
                # Both hunt and counts threshold at rm2; the true max always
                # passes, and any extra match (tie or near-tie) is flagged.
                rm2 = spool.tile([128, 1], f32, tag="rm2")
                nc.vector.tensor_scalar(
                    A[:, 2048:3072], fmax, 1.0 - DELTA, None, Alu.mult, Alu.max,
                    accum_out=rm2[:],
                )
                # offset hunt on folded fmax
                nc.vector.scalar_tensor_tensor(
                    out=A[:, 3072:4096],
                    in0=fmax,
                    scalar=rm2[:],
                    in1=iota_sb[:],
                    op0=Alu.is_ge,
                    op1=Alu.mult,
                    accum_out=off_st[:, m:m + 1],
                )
                # per-1024-chunk near-tie counts
                for f in range(NHC):
                    nc.vector.tensor_scalar(
                        A[:, 4096:5120],
                        E[:, f * HW_:(f + 1) * HW_],
                        rm2[:],
                        None,
                        Alu.is_ge,
                        Alu.add,
                        accum_out=cnt_st[:, m * NHC + f : m * NHC + f + 1],
                    )

            nc.sync.dma_start(rs_dram[:], rs_st[:])
            nc.sync.dma_start(cnt_dram[:], cnt_st[:])
            nc.sync.dma_start(off_dram[:], off_st[:])

    nc.compile()
    return nc


def _get_nc():
    if "nc" not in _CACHE:
        _CACHE["nc"] = _build()
    return _CACHE["nc"]


def _tiles(x_blk):
    """[1536, 128] rows -> [128, 1536] partition-major tile layout."""
    return np.ascontiguousarray(
        x_blk.reshape(RT, 128, D).transpose(1, 0, 2).reshape(128, RT * D)
    )


def _unstage(a):
    """[128, RT] staged column-per-row-tile -> [1536] block vector."""
    return np.ascontiguousarray(a.T).reshape(BLK)


def kernel(desc_0, desc_1, corr_0, corr_1, logits_0, logits_1):
    from concourse import bass_utils

    nc = _get_nc()

    d0 = np.asarray(desc_0, dtype=np.float32)
    d1 = np.asarray(desc_1, dtype=np.float32)
    c0 = np.asarray(corr_0)
    c1 = np.asarray(corr_1)
    l0g = np.asarray(logits_0, dtype=np.float32)
    l1g = np.asarray(logits_1, dtype=np.float32)

    d0T = np.ascontiguousarray(d0.T)
    d1T = np.ascontiguousarray(d1.T)
    i0 = np.clip(c0, 0, None).astype(np.int64)
    i1 = np.clip(c1, 0, None).astype(np.int64)
    G0 = d1[i0]   # [N, D]
    G1 = d0[i1]
    # Offset ramp: single match -> accum in [1025, 2048]; k>=2 matches sum to
    # >= 2051, disjoint, so multi-match ambiguity is detectable on the host.
    iota = np.broadcast_to(
        (np.arange(1, HW_ + 1, dtype=np.float16) + np.float16(1024.0))[None, :],
        (128, HW_),
    ).copy()

    in_maps = []
    for c in range(NCORES):
        sl = slice(c * BLK, (c + 1) * BLK)
        in_maps.append({
            "d0T": d0T,
            "d1T": d1T,
            "d0Tblk": np.ascontiguousarray(d0T[:, sl]),
            "d1Tblk": np.ascontiguousarray(d1T[:, sl]),
            "iota": iota,
        })

    import os
    res = bass_utils.run_bass_kernel_spmd(
        nc, in_maps, core_ids=list(range(NCORES)),
        trace=bool(os.environ.get("KERNEL_TRACE")),
    )
    _CACHE["last_res"] = res
    outs = res.results

    rs = {0: [], 1: []}
    best = {0: [], 1: []}
    fixup = {0: [], 1: []}   # global rows needing exact recompute
    for c in range(NCORES):
        o = outs[c]
        for d in (0, 1):
            r6 = o[f"rs{d}"].reshape(128, RT, NG).sum(axis=2, dtype=np.float64)
            rs[d].append(np.ascontiguousarray(r6.T).reshape(BLK))
            cnt = o[f"cnt{d}"].reshape(128, RT, NHC)
            off = o[f"off{d}"]                      # [128, RT]
            total = cnt.sum(axis=2)
            wc = np.argmax(cnt, axis=2)             # winning chunk (count==1 rows)
            oin = off - 1025.0                      # in-chunk offset
            b = wc.astype(np.int64) * HW_ + np.clip(oin, 0, HW_ - 1).astype(np.int64)
            best[d].append(_unstage(b))
            bad = (total != 1.0) | (off < 1024.5) | (off > 2048.5)
            for r in np.nonzero(_unstage(bad))[0]:
                fixup[d].append(c * BLK + int(r))

    rs0 = np.concatenate(rs[0]); rs1 = np.concatenate(rs[1])
    pos_0 = (np.float32(10.0) * np.einsum('ij,ij->i', d0, G0)).astype(np.float32)
    pos_1 = (np.float32(10.0) * np.einsum('ij,ij->i', d1, G1)).astype(np.float32)
    best_0 = np.concatenate(best[0]); best_1 = np.concatenate(best[1])

    # Exact fixup: rows where the max was ambiguous at fp16/f32r precision.
    # Recompute those rows in f64 and take the first argmax (jnp semantics).
    if fixup[0]:
        rows = np.asarray(fixup[0], dtype=np.int64)
        sl = d1.astype(np.float64) @ d0[rows].astype(np.float64).T   # [N, R]
        best_0[rows] = np.argmax(sl, axis=0)
    if fixup[1]:
        rows = np.asarray(fixup[1], dtype=np.int64)
        sl = d0.astype(np.float64) @ d1[rows].astype(np.float64).T
        best_1[rows] = np.argmax(sl, axis=0)

    lse_0 = np.log(rs0).astype(np.float32)
    lse_1 = np.log(rs1).astype(np.float32)

    m0 = c0 >= 0
    m1 = c1 >= 0
    l0 = np.where(m0, lse_0 - pos_0, np.float32(0.0)).astype(np.float32)
    l1 = np.where(m1, lse_1 - pos_1, np.float32(0.0)).astype(np.float32)
    n0 = max(int(m0.sum()), 1)
    n1 = max(int(m1.sum()), 1)
    loss_0 = np.float32(l0.sum(dtype=np.float32) / np.float32(n0))
    loss_1 = np.float32(l1.sum(dtype=np.float32) / np.float32(n1))

    best_0 = np.clip(best_0, 0, N - 1)
    best_1 = np.clip(best_1, 0, N - 1)
    _CACHE["dbg"] = dict(best_0=best_0, best_1=best_1, lse_0=lse_0, lse_1=lse_1,
                         n_fixup=(len(fixup[0]), len(fixup[1])))
    mutual = best_1[best_0] == np.arange(N)
    kp0 = l0g >= 0.0
    kp1 = l1g >= 0.0
    predicted = mutual & kp0 & kp1[best_0]
    correct = (best_0 == c0) & m0
    tp = int((correct & predicted).sum())
    precision = np.float32(np.float32(tp) / np.float32(max(int(predicted.sum()), 1)))
    recall = np.float32(np.float32(tp) / np.float32(n0))

    return loss_0, loss_1, precision, recall
